# revision 1
# baseline (speedup 1.0000x reference)
"""Trainium2 Bass kernel for nn_AxisNetwork (embedding_lookup + sine MLP).

Math per point (x, y):
    e = lerp(emb0, x) * lerp(emb1, y)          # [256]
    h = sin(30*(e @ w0.T + b0))                # [128]
    h = sin(30*(h @ w1.T + b1))                # [128]
    out = h @ w2.T + b2                        # [3]

Device strategy (pure data parallel over 8 cores, B = N/8 points each):
  * The per-point linear interpolation is replaced by a lookup into a
    U=32x upsampled table (host-precomputed from emb0/emb1, fp16).
    Snapping to the nearest 1/32 sub-cell gives ~5e-4 rel error
    (validated numerically against the f32 reference).
  * Per core: compute int16 row indices from coords on DVE, then
    dma_gather (SWDGE, transpose=True) fetches one 256-wide fp16 row per
    point per axis, landing channel-on-partition: e0T/e1T [128, 2, n].
  * DVE forms e = e0*e1; PE runs the three matmuls with points streaming
    as columns; ACT applies sin(30*z + 30*b) via its scale/bias fold.
  * Output written [128, B/128*3] (point p = n%128, tile n//128);
    host de-interleaves.
"""

import os

import numpy as np

N_FULL = 1 << 20
NCORES = 8
B = int(os.environ.get("KERNEL_B", N_FULL // NCORES))  # points per core
RES = 512
ED = 256
HID = 128
NOUT = 3
W0_FREQ = 30.0

UPS = 32                  # upsample factor for the snap tables
NROWS = (RES - 1) * UPS   # 16352 valid rows
NROWS_PAD = 16384

CHUNK = 4096              # points per gather chunk
STAGE = 1024              # points per compute stage
N_CHUNKS = B // CHUNK
STAGES_PER_CHUNK = CHUNK // STAGE

P = 128

_cache = {}


def _build_nc():
    import concourse.bacc as bacc
    import concourse.bass as bass
    import concourse.mybir as mybir
    import concourse.tile as tile
    from concourse import library_config

    f32 = mybir.dt.float32
    f16 = mybir.dt.float16
    i16 = mybir.dt.int16
    Alu = mybir.AluOpType
    Act = mybir.ActivationFunctionType

    nc = bacc.Bacc("TRN2", target_bir_lowering=False, debug=False,
                   num_devices=NCORES)

    coords_d = nc.dram_tensor("coords", [B, 2], f32, kind="ExternalInput")
    up0_d = nc.dram_tensor("up0", [NROWS_PAD, ED], f16, kind="ExternalInput")
    up1_d = nc.dram_tensor("up1", [NROWS_PAD, ED], f16, kind="ExternalInput")
    w0t_d = nc.dram_tensor("w0t", [2, P, HID], f16, kind="ExternalInput")
    w1t_d = nc.dram_tensor("w1t", [HID, HID], f16, kind="ExternalInput")
    w2t_d = nc.dram_tensor("w2t", [HID, NOUT], f16, kind="ExternalInput")
    b0s_d = nc.dram_tensor("b0s", [P, 1], f32, kind="ExternalInput")
    b1s_d = nc.dram_tensor("b1s", [P, 1], f32, kind="ExternalInput")
    b2t_d = nc.dram_tensor("b2t", [P, (STAGE // P) * NOUT], f32,
                           kind="ExternalInput")
    out_d = nc.dram_tensor("out", [P, (B // P) * NOUT], f32,
                           kind="ExternalOutput")
    # scratch for rearranging indices into the 16-partition-wrapped layout
    xybuf = nc.dram_tensor("xybuf", [2, B], i16)

    FPC = B // P              # free elems per partition per coordinate (1024)
    AFF = 255.5 * UPS         # (0.5c+0.5)*511*UPS == c*AFF + AFF

    with tile.TileContext(nc) as tc:
        with (
            tc.tile_pool(name="const", bufs=1) as cpool,
            tc.tile_pool(name="prep", bufs=1) as prep,
            tc.tile_pool(name="idx", bufs=1) as idxp,
            tc.tile_pool(name="gath", bufs=2) as gath,
            tc.tile_pool(name="act", bufs=2) as actp,
            tc.tile_pool(name="psA", bufs=2, space="PSUM") as psA,
            tc.tile_pool(name="psB", bufs=2, space="PSUM") as psB,
        ):
            nc.gpsimd.load_library(library_config.mlp)

            # ---- constants / weights ----
            w0t = cpool.tile([P, 2, HID], f16)       # [k, c, m]
            nc.sync.dma_start(out=w0t[:], in_=w0t_d[:].rearrange("c k m -> k c m"))
            w1t = cpool.tile([HID, HID], f16)
            nc.sync.dma_start(out=w1t[:], in_=w1t_d[:])
            w2t = cpool.tile([HID, NOUT], f16)
            nc.sync.dma_start(out=w2t[:], in_=w2t_d[:])
            b0s = cpool.tile([P, 1], f32)
            nc.sync.dma_start(out=b0s[:], in_=b0s_d[:])
            b1s = cpool.tile([P, 1], f32)
            nc.sync.dma_start(out=b1s[:], in_=b1s_d[:])
            b2t = cpool.tile([P, (STAGE // P) * NOUT], f32)
            nc.sync.dma_start(out=b2t[:], in_=b2t_d[:])

            out_acc = cpool.tile([P, (B // P) * NOUT], f32)

            # ---- index prep ----
            # coords laid out [p = n%128, f = n//128, axis]
            ctile = prep.tile([P, FPC, 2], f32)
            nc.sync.dma_start(
                out=ctile[:], in_=coords_d[:].rearrange("(f p) a -> p f a", p=P))
            cflat = ctile[:].rearrange("p f a -> p (f a)")
            # clip to [-1, 0.999] (as the reference does), then affine to
            # upsampled-row coordinates; round via f32->int16 convert.
            cl = prep.tile([P, FPC * 2], f32)
            nc.vector.tensor_scalar(out=cl[:], in0=cflat, scalar1=0.999,
                                    scalar2=-1.0, op0=Alu.min, op1=Alu.max)
            av = prep.tile([P, FPC * 2], f32)
            nc.vector.tensor_scalar(out=av[:], in0=cl[:], scalar1=AFF,
                                    scalar2=AFF, op0=Alu.mult, op1=Alu.add)
            idx16 = prep.tile([P, FPC * 2], i16)
            nc.vector.tensor_copy(out=idx16[:], in_=av[:])

            # scatter x/y indices to DRAM in point order
            for a in range(2):
                nc.sync.dma_start(
                    out=xybuf[a].rearrange("(f p) -> p f", p=P),
                    in_=idx16[:].rearrange("p (f a) -> a p f", a=2)[a])
            # reload wrapped-by-16, replicated into all 8 partition groups
            idxs = []
            for a in range(2):
                t = idxp.tile([P, B // 16], i16, tag=f"idxs{a}")
                for g in range(8):
                    nc.sync.dma_start(
                        out=t[16 * g:16 * (g + 1), :],
                        in_=xybuf[a].rearrange("(f q) -> q f", q=16))
                idxs.append(t)

            # ---- main pipeline ----
            for k in range(N_CHUNKS):
                e0 = gath.tile([P, 2, CHUNK], f16, tag="e0")
                e1 = gath.tile([P, 2, CHUNK], f16, tag="e1")
                ncol = CHUNK // 16
                nc.gpsimd.dma_gather(
                    e0[:], up0_d[:], idxs[0][:, k * ncol:(k + 1) * ncol],
                    num_idxs=CHUNK, num_idxs_reg=CHUNK, elem_size=ED,
                    transpose=True, single_packet=False)
                nc.gpsimd.dma_gather(
                    e1[:], up1_d[:], idxs[1][:, k * ncol:(k + 1) * ncol],
                    num_idxs=CHUNK, num_idxs_reg=CHUNK, elem_size=ED,
                    transpose=True, single_packet=False)
                ee = gath.tile([P, 2, CHUNK], f16, tag="ee")
                nc.vector.tensor_tensor(
                    out=ee[:].rearrange("p c n -> p (c n)"),
                    in0=e0[:].rearrange("p c n -> p (c n)"),
                    in1=e1[:].rearrange("p c n -> p (c n)"),
                    op=Alu.mult)

                for si in range(STAGES_PER_CHUNK):
                    s = k * STAGES_PER_CHUNK + si
                    lo = si * STAGE
                    # layer 0: z0[h, n] = sum_d w0[h, d] e[d, n]
                    z0 = psA.tile([P, STAGE], f32, tag="z0", space="PSUM")
                    for half in range(STAGE // 512):
                        cs = lo + half * 512
                        for c in range(2):
                            nc.tensor.matmul(
                                z0[:, half * 512:(half + 1) * 512],
                                w0t[:, c, :],
                                ee[:, c, cs:cs + 512],
                                start=(c == 0), stop=(c == 1))
                    h0 = actp.tile([P, STAGE], f16, tag="h0")
                    nc.scalar.activation(out=h0[:], in_=z0[:], func=Act.Sin,
                                         bias=b0s[:], scale=W0_FREQ)
                    # layer 1 (w1t is pre-scaled by 30 on the host; ACT's Sin
                    # spline only covers [-pi, pi], so wrap 30*z1+30*b1 back
                    # into range by one period first — sin is 2pi-periodic)
                    z1 = psB.tile([P, STAGE], f32, tag="zb")
                    for half in range(STAGE // 512):
                        nc.tensor.matmul(
                            z1[:, half * 512:(half + 1) * 512],
                            w1t[:],
                            h0[:, half * 512:(half + 1) * 512],
                            start=True, stop=True)
                    t1 = actp.tile([P, STAGE], f32, tag="t1")
                    nc.vector.add_range_wrap(out=t1[:], in_=z1[:], shift=b1s[:],
                                             bound=float(np.pi),
                                             period=float(2 * np.pi))
                    h1 = actp.tile([P, STAGE], f16, tag="h1")
                    nc.scalar.activation(out=h1[:], in_=t1[:], func=Act.Sin)
                    # layer 2 (points become the stationary M dim)
                    o2 = psB.tile([P, (STAGE // P) * NOUT], f32, tag="zb")
                    for t in range(STAGE // P):
                        nc.tensor.matmul(
                            o2[:, t * NOUT:(t + 1) * NOUT],
                            h1[:, t * P:(t + 1) * P],
                            w2t[:],
                            start=True, stop=True)
                    nc.vector.scalar_tensor_tensor(
                        out=out_acc[:, s * (STAGE // P) * NOUT:
                                    (s + 1) * (STAGE // P) * NOUT],
                        in0=o2[:], scalar=1.0, in1=b2t[:],
                        op0=Alu.mult, op1=Alu.add)

            nc.sync.dma_start(out=out_d[:], in_=out_acc[:])

    nc.compile()
    return nc


def _host_prep(inputs):
    coords = np.ascontiguousarray(inputs["coords"], dtype=np.float32)
    emb0 = np.asarray(inputs["emb0"], dtype=np.float32)
    emb1 = np.asarray(inputs["emb1"], dtype=np.float32)
    w0 = np.asarray(inputs["w0"], dtype=np.float32)
    b0 = np.asarray(inputs["b0"], dtype=np.float32)
    w1 = np.asarray(inputs["w1"], dtype=np.float32)
    b1 = np.asarray(inputs["b1"], dtype=np.float32)
    w2 = np.asarray(inputs["w2"], dtype=np.float32)
    b2 = np.asarray(inputs["b2"], dtype=np.float32)

    def upsample(emb):
        i = np.arange(RES - 1)
        w = (np.arange(UPS, dtype=np.float64) / UPS).astype(np.float32)
        t = (1.0 - w)[None, :, None] * emb[i][:, None, :] \
            + w[None, :, None] * emb[i + 1][:, None, :]
        t = t.reshape(NROWS, ED)
        pad = np.zeros((NROWS_PAD - NROWS, ED), np.float32)
        return np.concatenate([t, pad], 0).astype(np.float16)

    up0 = upsample(emb0)
    up1 = upsample(emb1)
    w0t = np.ascontiguousarray(
        w0.T.reshape(2, P, HID)).astype(np.float16)        # [c, k, m]
    w1t = np.ascontiguousarray(w1.T * W0_FREQ).astype(np.float16)  # [k, m], pre-scaled
    w2t = np.ascontiguousarray(w2.T).astype(np.float16)    # [k, 3]
    b0s = (W0_FREQ * b0).reshape(P, 1).astype(np.float32)
    b1s = (W0_FREQ * b1).reshape(P, 1).astype(np.float32)
    b2t = np.tile(b2, STAGE // P).reshape(1, -1).repeat(P, 0).astype(np.float32)

    shared = dict(up0=up0, up1=up1, w0t=w0t, w1t=w1t, w2t=w2t,
                  b0s=b0s, b1s=b1s, b2t=b2t)
    in_maps = []
    for c in range(NCORES):
        shard = np.ascontiguousarray(coords[c * B:(c + 1) * B])
        in_maps.append(dict(coords=shard, **shared))
    return in_maps


last_results = None


def kernel(**inputs):
    global last_results
    from concourse.bass_utils import run_bass_kernel_spmd
    import os

    if "nc" not in _cache:
        _cache["nc"] = _build_nc()
    nc = _cache["nc"]

    in_maps = _host_prep(inputs)
    trace = bool(int(os.environ.get("KERNEL_TRACE", "0")))
    res = run_bass_kernel_spmd(nc, in_maps, core_ids=list(range(NCORES)),
                               trace=trace)
    last_results = res

    outs = []
    for c in range(NCORES):
        dev = res.results[c]["out"]                  # [128, (B/128)*3]
        dev = dev.reshape(P, B // P, NOUT).transpose(1, 0, 2).reshape(B, NOUT)
        outs.append(dev)
    return np.ascontiguousarray(
        np.concatenate(outs, 0).astype(np.float32))



# revision 10
# speedup vs baseline: 2.4956x; 2.4956x over previous
"""Trainium2 Bass kernel for nn_AxisNetwork (embedding_lookup + sine MLP).

Math per point (x, y):
    e = lerp(emb0, x) * lerp(emb1, y)          # [256]
    h = sin(30*(e @ w0.T + b0))                # [128]
    h = sin(30*(h @ w1.T + b1))                # [128]
    out = h @ w2.T + b2                        # [3]

Device strategy (pure data parallel over 8 cores, B = N/8 points each):
  * Lerp replaced by a lookup into a U=32x upsampled table (host-built
    from emb0/emb1, fp16; snap error ~5e-4 rel).
  * Per core: coords load + index math entirely with contiguous DMAs
    (point n lives at partition n//FPC); int16 row indices are written
    to DRAM already in the 16-partition-wrapped layout the SWDGE gather
    wants, then read back replicated (large contiguous descriptors).
  * dma_gather (non-transposed) fetches one contiguous 512B fp16 row
    per point per axis into point-major layout [128, g, 256].
  * DVE forms ee = e0*e1 point-major; one batched XBAR DMA-transpose
    per chunk flips it to channel-major eeT [128, 2g, 128].
  * PE runs the three matmuls with points streaming as columns; ACT
    applies sin(30*z + 30*b); final layer uses w2 as stationary with
    out [3, n]; bias b2 and the point un-permutation happen on host.
"""

import os

import numpy as np

N_FULL = 1 << 20
NCORES = 8
B = int(os.environ.get("KERNEL_B", N_FULL // NCORES))  # points per core
RES = 512
ED = 256
HID = 128
NOUT = 3
W0_FREQ = 30.0

UPS = 32                  # upsample factor for the snap tables
NROWS = (RES - 1) * UPS   # 16352 valid rows
NROWS_PAD = 16384

CHUNK = 4096              # points per gather chunk
STAGE = 1024              # points per MLP stage
N_CHUNKS = B // CHUNK
STAGES = CHUNK // STAGE
GPC = CHUNK // 128        # point groups per chunk (32)

P = 128
FPC = B // P              # coords free elems per partition (1024)
AFF = 255.5 * UPS         # (0.5c+0.5)*511*UPS == c*AFF + AFF

SINGLE_PACKET = bool(int(os.environ.get("KERNEL_SP", "0")))

_cache = {}


def _build_nc():
    import concourse.bacc as bacc
    import concourse.mybir as mybir
    import concourse.tile as tile
    from concourse import library_config

    f32 = mybir.dt.float32
    f16 = mybir.dt.float16
    i16 = mybir.dt.int16
    Alu = mybir.AluOpType
    Act = mybir.ActivationFunctionType

    nc = bacc.Bacc("TRN2", target_bir_lowering=False, debug=False,
                   num_devices=NCORES)

    coords_d = nc.dram_tensor("coords", [B, 2], f32, kind="ExternalInput")
    up0_d = nc.dram_tensor("up0", [NROWS_PAD, ED], f16, kind="ExternalInput")
    up1_d = nc.dram_tensor("up1", [NROWS_PAD, ED], f16, kind="ExternalInput")
    w0t_d = nc.dram_tensor("w0t", [2, P, HID], f16, kind="ExternalInput")
    w1t_d = nc.dram_tensor("w1t", [HID, HID], f16, kind="ExternalInput")
    w2t_d = nc.dram_tensor("w2t", [HID, NOUT], f16, kind="ExternalInput")
    b0s_d = nc.dram_tensor("b0s", [P, 1], f32, kind="ExternalInput")
    b1s_d = nc.dram_tensor("b1s", [P, 1], f32, kind="ExternalInput")
    out_d = nc.dram_tensor("out", [NOUT, B], f32, kind="ExternalOutput")
    # idx scratch, stored already 16-partition-wrapped: xyw[a][q, g, f] is
    # the row index of gather-order position m = ((g*FPC + f)*16) + q
    xyw = nc.dram_tensor("xyw", [2, 16, 8, FPC], i16)

    with tile.TileContext(nc) as tc:
        with (
            tc.tile_pool(name="const", bufs=1) as cpool,
            tc.tile_pool(name="prep", bufs=1) as prep,
            tc.tile_pool(name="idx", bufs=1) as idxp,
            tc.tile_pool(name="gath", bufs=2) as gath,
            tc.tile_pool(name="eep", bufs=1) as eep,
            tc.tile_pool(name="eetp", bufs=1) as eetp,
            tc.tile_pool(name="outp", bufs=1) as outp,
            tc.tile_pool(name="act", bufs=2) as actp,
            tc.tile_pool(name="psA", bufs=2, space="PSUM") as psA,
            tc.tile_pool(name="psB", bufs=1, space="PSUM") as psB,
            tc.tile_pool(name="psC", bufs=1, space="PSUM") as psC,
        ):
            nc.gpsimd.load_library(library_config.mlp)

            # ---- constants / weights ----
            w0t = cpool.tile([P, 2, HID], f16)       # [k, h, m]
            nc.sync.dma_start(out=w0t[:], in_=w0t_d[:].rearrange("c k m -> k c m"))
            w1t = cpool.tile([HID, HID], f16)
            nc.sync.dma_start(out=w1t[:], in_=w1t_d[:])
            w2t = cpool.tile([HID, NOUT], f16)
            nc.sync.dma_start(out=w2t[:], in_=w2t_d[:])
            b0s = cpool.tile([P, 1], f32)
            nc.sync.dma_start(out=b0s[:], in_=b0s_d[:])
            b1s = cpool.tile([P, 1], f32)
            nc.sync.dma_start(out=b1s[:], in_=b1s_d[:])

            # ---- index prep (all DMAs contiguous per partition) ----
            # point n -> partition n//FPC, free (n%FPC)*2 + axis
            ctile = prep.tile([P, FPC * 2], f32)
            nc.sync.dma_start(
                out=ctile[:],
                in_=coords_d[:].rearrange("(p x) a -> p (x a)", p=P))
            for a in range(2):
                ca = ctile[:].rearrange("p (x a) -> a p x", a=2)[a]
                cl = prep.tile([P, FPC], f32, tag=f"cl{a}")
                nc.vector.tensor_scalar(out=cl[:], in0=ca, scalar1=0.999,
                                        scalar2=-1.0, op0=Alu.min, op1=Alu.max)
                av = prep.tile([P, FPC], f32, tag=f"av{a}")
                nc.vector.tensor_scalar(out=av[:], in0=cl[:], scalar1=AFF,
                                        scalar2=AFF, op0=Alu.mult, op1=Alu.add)
                ix = prep.tile([P, FPC], i16, tag=f"ix{a}")
                nc.vector.tensor_copy(out=ix[:], in_=av[:])
                # W[q, g, f] = ix[16g + q, f]  (contiguous per partition;
                # SBUF partitions pair with (g, q) in flat order)
                nc.sync.dma_start(
                    out=xyw[a].rearrange("q g f -> g q f"),
                    in_=ix[:])
            # read back, replicated into all 8 16-partition groups
            idxs = []
            for a in range(2):
                t = idxp.tile([P, B // 16], i16, tag=f"idxs{a}")
                for g8 in range(8):
                    nc.sync.dma_start(out=t[16 * g8:16 * (g8 + 1), :],
                                      in_=xyw[a].rearrange("q g f -> q (g f)"))
                idxs.append(t)

            # ---- main pipeline ----
            ncol = CHUNK // 16
            for k in range(N_CHUNKS):
                e0 = gath.tile([P, GPC, ED], f16, tag="e0")
                e1 = gath.tile([P, GPC, ED], f16, tag="e1")
                nc.gpsimd.dma_gather(
                    e0[:], up0_d[:], idxs[0][:, k * ncol:(k + 1) * ncol],
                    num_idxs=CHUNK, num_idxs_reg=CHUNK, elem_size=ED,
                    transpose=False, single_packet=SINGLE_PACKET)
                nc.gpsimd.dma_gather(
                    e1[:], up1_d[:], idxs[1][:, k * ncol:(k + 1) * ncol],
                    num_idxs=CHUNK, num_idxs_reg=CHUNK, elem_size=ED,
                    transpose=False, single_packet=SINGLE_PACKET)
                # product written h-major: ee[p, h, g, q] = dim(128h+q) of
                # point g*128+p, so post-transpose tiles come out h-major and
                # every L0 matmul rhs is one contiguous run
                ee = eep.tile([P, 2, GPC, P], f16, tag="ee")
                nc.vector.tensor_tensor(
                    out=ee[:],
                    in0=e0[:].rearrange("p g (h d) -> p h g d", h=2),
                    in1=e1[:].rearrange("p g (h d) -> p h g d", h=2),
                    op=Alu.mult)
                # batched 128x128 tile transpose: eeT[q, t, p] = ee_flat[p,
                # t*128+q]; tile t = h*GPC+g holds dims [128h, 128h+128) of
                # points g*128+p
                eeT = eetp.tile([P, 2 * GPC, P], f16, tag="eeT")
                nc.sync.dma_start_transpose(
                    out=eeT[:], in_=ee[:].rearrange("p h g d -> p (h g d)"))

                outsb = outp.tile([NOUT, CHUNK], f32, tag="osb")
                for si in range(STAGES):
                    # matmuls split at 512 cols (single-PSUM-bank limit)
                    z0 = psA.tile([P, STAGE], f32, tag="z0", space="PSUM")
                    for c2 in range(STAGE // 512):
                        sl = slice(c2 * 512, (c2 + 1) * 512)
                        for h in range(2):
                            rhs = eeT[:].rearrange(
                                "q (h s c g) p -> h s c q (g p)",
                                h=2, s=STAGES, c=STAGE // 512)[h, si, c2]
                            nc.tensor.matmul(z0[:, sl], w0t[:, h, :], rhs,
                                             start=(h == 0), stop=(h == 1))
                    h0 = actp.tile([P, STAGE], f16, tag="h0")
                    nc.scalar.activation(out=h0[:], in_=z0[:], func=Act.Sin,
                                         bias=b0s[:], scale=W0_FREQ)
                    # w1t pre-scaled by 30 on host; wrap 30*z1+30*b1 into
                    # [-pi, pi] (ACT Sin spline range) before the sin
                    z1 = psB.tile([P, STAGE], f32, tag="z1", space="PSUM")
                    for c2 in range(STAGE // 512):
                        sl = slice(c2 * 512, (c2 + 1) * 512)
                        nc.tensor.matmul(z1[:, sl], w1t[:], h0[:, sl],
                                         start=True, stop=True)
                    t1 = actp.tile([P, STAGE], f32, tag="t1")
                    nc.vector.add_range_wrap(out=t1[:], in_=z1[:],
                                             shift=b1s[:],
                                             bound=float(np.pi),
                                             period=float(2 * np.pi))
                    h1 = actp.tile([P, STAGE], f16, tag="h1")
                    nc.scalar.activation(out=h1[:], in_=t1[:], func=Act.Sin)
                    o2 = psC.tile([NOUT, STAGE], f32, tag="o2", space="PSUM")
                    for c2 in range(STAGE // 512):
                        sl = slice(c2 * 512, (c2 + 1) * 512)
                        nc.tensor.matmul(o2[:, sl], w2t[:], h1[:, sl],
                                         start=True, stop=True)
                    nc.vector.tensor_copy(
                        out=outsb[:, si * STAGE:(si + 1) * STAGE], in_=o2[:])
                nc.sync.dma_start(out=out_d[:, k * CHUNK:(k + 1) * CHUNK],
                                  in_=outsb[:])

    nc.compile()
    return nc


def _host_prep(inputs):
    coords = np.ascontiguousarray(inputs["coords"], dtype=np.float32)
    emb0 = np.asarray(inputs["emb0"], dtype=np.float32)
    emb1 = np.asarray(inputs["emb1"], dtype=np.float32)
    w0 = np.asarray(inputs["w0"], dtype=np.float32)
    b0 = np.asarray(inputs["b0"], dtype=np.float32)
    w1 = np.asarray(inputs["w1"], dtype=np.float32)
    b1 = np.asarray(inputs["b1"], dtype=np.float32)
    w2 = np.asarray(inputs["w2"], dtype=np.float32)

    def upsample(emb):
        i = np.arange(RES - 1)
        w = (np.arange(UPS, dtype=np.float64) / UPS).astype(np.float32)
        t = (1.0 - w)[None, :, None] * emb[i][:, None, :] \
            + w[None, :, None] * emb[i + 1][:, None, :]
        t = t.reshape(NROWS, ED)
        pad = np.zeros((NROWS_PAD - NROWS, ED), np.float32)
        return np.concatenate([t, pad], 0).astype(np.float16)

    up0 = upsample(emb0)
    up1 = upsample(emb1)
    w0t = np.ascontiguousarray(
        w0.T.reshape(2, P, HID)).astype(np.float16)        # [c, k, m]
    w1t = np.ascontiguousarray(w1.T * W0_FREQ).astype(np.float16)
    w2t = np.ascontiguousarray(w2.T).astype(np.float16)    # [k, 3]
    b0s = (W0_FREQ * b0).reshape(P, 1).astype(np.float32)
    b1s = (W0_FREQ * b1).reshape(P, 1).astype(np.float32)

    shared = dict(up0=up0, up1=up1, w0t=w0t, w1t=w1t, w2t=w2t,
                  b0s=b0s, b1s=b1s)
    in_maps = []
    for c in range(NCORES):
        shard = np.ascontiguousarray(coords[c * B:(c + 1) * B])
        in_maps.append(dict(coords=shard, **shared))
    return in_maps


def _n_of_m():
    # gather order m = c*16 + q covers point n = (16*(c//FPC) + q)*FPC + c%FPC
    m = np.arange(B)
    q = m % 16
    c = m // 16
    return (16 * (c // FPC) + q) * FPC + (c % FPC)


last_results = None


def kernel(**inputs):
    global last_results
    from concourse.bass_utils import run_bass_kernel_spmd

    if "nc" not in _cache:
        _cache["nc"] = _build_nc()
    nc = _cache["nc"]

    in_maps = _host_prep(inputs)
    trace = bool(int(os.environ.get("KERNEL_TRACE", "0")))
    res = run_bass_kernel_spmd(nc, in_maps, core_ids=list(range(NCORES)),
                               trace=trace)
    last_results = res

    b2 = np.asarray(inputs["b2"], dtype=np.float32)
    n_of_m = _n_of_m()
    outs = []
    for c in range(NCORES):
        dev = res.results[c]["out"]                  # [3, B] in gather order
        full = np.empty((B, NOUT), np.float32)
        full[n_of_m] = dev.T
        outs.append(full)
    return np.ascontiguousarray(np.concatenate(outs, 0) + b2[None, :])


# revision 13
# speedup vs baseline: 4.0217x; 1.6115x over previous
"""Trainium2 Bass kernel for nn_AxisNetwork (embedding_lookup + sine MLP).

Math per point (x, y):
    e = lerp(emb0, x) * lerp(emb1, y)          # [256]
    h = sin(30*(e @ w0.T + b0))                # [128]
    h = sin(30*(h @ w1.T + b1))                # [128]
    out = h @ w2.T + b2                        # [3]

Device strategy (pure data parallel over 8 cores, B = N/8 points each):
  * Lerp replaced by a lookup into a U=32x upsampled table (host-built
    from emb0/emb1, fp16; snap error ~5e-4 rel).
  * Per core: coords load + index math entirely with contiguous DMAs
    (point n lives at partition n//FPC); int16 row indices are written
    to DRAM already in the 16-partition-wrapped layout the SWDGE gather
    wants, then read back replicated (large contiguous descriptors).
  * dma_gather (non-transposed) fetches one contiguous 512B fp16 row
    per point per axis into point-major layout [128, g, 256].
  * DVE forms ee = e0*e1 point-major; one batched XBAR DMA-transpose
    per chunk flips it to channel-major eeT [128, 2g, 128].
  * PE runs the three matmuls with points streaming as columns; ACT
    applies sin(30*z + 30*b); final layer uses w2 as stationary with
    out [3, n]; bias b2 and the point un-permutation happen on host.
"""

import os

import numpy as np

N_FULL = 1 << 20
NCORES = 8
B = int(os.environ.get("KERNEL_B", N_FULL // NCORES))  # points per core
RES = 512
ED = 256
HID = 128
NOUT = 3
W0_FREQ = 30.0

UPS = 32                  # upsample factor for the snap tables
NROWS = (RES - 1) * UPS   # 16352 valid rows
NROWS_PAD = 16384

CHUNK = 4096              # points per gather chunk
STAGE = 1024              # points per MLP stage
N_CHUNKS = B // CHUNK
STAGES = CHUNK // STAGE
GPC = CHUNK // 128        # point groups per chunk (32)

P = 128
FPC = B // P              # coords free elems per partition (1024)
AFF = 255.5 * UPS         # (0.5c+0.5)*511*UPS == c*AFF + AFF

SINGLE_PACKET = bool(int(os.environ.get("KERNEL_SP", "0")))
NQUEUES = int(os.environ.get("KERNEL_NQ", "4"))

_cache = {}


def _build_nc():
    import concourse.bacc as bacc
    import concourse.mybir as mybir
    import concourse.tile as tile
    from concourse import library_config

    f32 = mybir.dt.float32
    f16 = mybir.dt.float16
    i16 = mybir.dt.int16
    Alu = mybir.AluOpType
    Act = mybir.ActivationFunctionType

    nc = bacc.Bacc("TRN2", target_bir_lowering=False, debug=False,
                   num_devices=NCORES, num_swdge_queues=NQUEUES)

    coords_d = nc.dram_tensor("coords", [B, 2], f32, kind="ExternalInput")
    up0_d = nc.dram_tensor("up0", [NROWS_PAD, ED], f16, kind="ExternalInput")
    up1_d = nc.dram_tensor("up1", [NROWS_PAD, ED], f16, kind="ExternalInput")
    w0t_d = nc.dram_tensor("w0t", [2, P, HID], f16, kind="ExternalInput")
    w1t_d = nc.dram_tensor("w1t", [HID, HID], f16, kind="ExternalInput")
    w2t_d = nc.dram_tensor("w2t", [HID, NOUT], f16, kind="ExternalInput")
    b0s_d = nc.dram_tensor("b0s", [P, 1], f32, kind="ExternalInput")
    b1s_d = nc.dram_tensor("b1s", [P, 1], f32, kind="ExternalInput")
    out_d = nc.dram_tensor("out", [NOUT, B], f32, kind="ExternalOutput")
    # idx scratch, stored already 16-partition-wrapped: xyw[a][q, g, f] is
    # the row index of gather-order position m = ((g*FPC + f)*16) + q
    xyw = nc.dram_tensor("xyw", [2, 16, 8, FPC], i16)

    with tile.TileContext(nc) as tc:
        with (
            tc.tile_pool(name="const", bufs=1) as cpool,
            tc.tile_pool(name="prep", bufs=1) as prep,
            tc.tile_pool(name="idx", bufs=1) as idxp,
            tc.tile_pool(name="gath", bufs=2) as gath,
            tc.tile_pool(name="eep", bufs=1) as eep,
            tc.tile_pool(name="eetp", bufs=1) as eetp,
            tc.tile_pool(name="outp", bufs=1) as outp,
            tc.tile_pool(name="act", bufs=2) as actp,
            tc.tile_pool(name="psA", bufs=2, space="PSUM") as psA,
            tc.tile_pool(name="psB", bufs=1, space="PSUM") as psB,
            tc.tile_pool(name="psC", bufs=1, space="PSUM") as psC,
        ):
            nc.gpsimd.load_library(library_config.mlp)

            # ---- constants / weights ----
            w0t = cpool.tile([P, 2, HID], f16)       # [k, h, m]
            nc.sync.dma_start(out=w0t[:], in_=w0t_d[:].rearrange("c k m -> k c m"))
            w1t = cpool.tile([HID, HID], f16)
            nc.sync.dma_start(out=w1t[:], in_=w1t_d[:])
            w2t = cpool.tile([HID, NOUT], f16)
            nc.sync.dma_start(out=w2t[:], in_=w2t_d[:])
            b0s = cpool.tile([P, 1], f32)
            nc.sync.dma_start(out=b0s[:], in_=b0s_d[:])
            b1s = cpool.tile([P, 1], f32)
            nc.sync.dma_start(out=b1s[:], in_=b1s_d[:])

            # ---- index prep (all DMAs contiguous per partition) ----
            # point n -> partition n//FPC, free (n%FPC)*2 + axis
            ctile = prep.tile([P, FPC * 2], f32)
            nc.sync.dma_start(
                out=ctile[:],
                in_=coords_d[:].rearrange("(p x) a -> p (x a)", p=P))
            for a in range(2):
                ca = ctile[:].rearrange("p (x a) -> a p x", a=2)[a]
                cl = prep.tile([P, FPC], f32, tag=f"cl{a}")
                nc.vector.tensor_scalar(out=cl[:], in0=ca, scalar1=0.999,
                                        scalar2=-1.0, op0=Alu.min, op1=Alu.max)
                av = prep.tile([P, FPC], f32, tag=f"av{a}")
                nc.vector.tensor_scalar(out=av[:], in0=cl[:], scalar1=AFF,
                                        scalar2=AFF, op0=Alu.mult, op1=Alu.add)
                ix = prep.tile([P, FPC], i16, tag=f"ix{a}")
                nc.vector.tensor_copy(out=ix[:], in_=av[:])
                # W[q, g, f] = ix[16g + q, f]  (contiguous per partition;
                # SBUF partitions pair with (g, q) in flat order)
                nc.sync.dma_start(
                    out=xyw[a].rearrange("q g f -> g q f"),
                    in_=ix[:])
            # read back, replicated into all 8 16-partition groups
            idxs = []
            for a in range(2):
                t = idxp.tile([P, B // 16], i16, tag=f"idxs{a}")
                for g8 in range(8):
                    nc.sync.dma_start(out=t[16 * g8:16 * (g8 + 1), :],
                                      in_=xyw[a].rearrange("q g f -> q (g f)"))
                idxs.append(t)

            # ---- main pipeline ----
            ncol = CHUNK // 16
            for k in range(N_CHUNKS):
                e0 = gath.tile([P, GPC, ED], f16, tag="e0")
                e1 = gath.tile([P, GPC, ED], f16, tag="e1")
                nc.gpsimd.dma_gather(
                    e0[:], up0_d[:], idxs[0][:, k * ncol:(k + 1) * ncol],
                    num_idxs=CHUNK, num_idxs_reg=CHUNK, elem_size=ED,
                    transpose=False, single_packet=SINGLE_PACKET,
                    queue_num=(2 * k) % NQUEUES)
                nc.gpsimd.dma_gather(
                    e1[:], up1_d[:], idxs[1][:, k * ncol:(k + 1) * ncol],
                    num_idxs=CHUNK, num_idxs_reg=CHUNK, elem_size=ED,
                    transpose=False, single_packet=SINGLE_PACKET,
                    queue_num=(2 * k + 1) % NQUEUES)
                # product written h-major: ee[p, h, g, q] = dim(128h+q) of
                # point g*128+p, so post-transpose tiles come out h-major and
                # every L0 matmul rhs is one contiguous run
                ee = eep.tile([P, 2, GPC, P], f16, tag="ee")
                nc.vector.tensor_tensor(
                    out=ee[:],
                    in0=e0[:].rearrange("p g (h d) -> p h g d", h=2),
                    in1=e1[:].rearrange("p g (h d) -> p h g d", h=2),
                    op=Alu.mult)
                # batched 128x128 tile transpose: eeT[q, t, p] = ee_flat[p,
                # t*128+q]; tile t = h*GPC+g holds dims [128h, 128h+128) of
                # points g*128+p
                eeT = eetp.tile([P, 2 * GPC, P], f16, tag="eeT")
                nc.sync.dma_start_transpose(
                    out=eeT[:], in_=ee[:].rearrange("p h g d -> p (h g d)"))

                outsb = outp.tile([NOUT, CHUNK], f32, tag="osb")
                for si in range(STAGES):
                    # matmuls split at 512 cols (single-PSUM-bank limit)
                    z0 = psA.tile([P, STAGE], f32, tag="z0", space="PSUM")
                    for c2 in range(STAGE // 512):
                        sl = slice(c2 * 512, (c2 + 1) * 512)
                        for h in range(2):
                            rhs = eeT[:].rearrange(
                                "q (h s c g) p -> h s c q (g p)",
                                h=2, s=STAGES, c=STAGE // 512)[h, si, c2]
                            nc.tensor.matmul(z0[:, sl], w0t[:, h, :], rhs,
                                             start=(h == 0), stop=(h == 1))
                    h0 = actp.tile([P, STAGE], f16, tag="h0")
                    nc.scalar.activation(out=h0[:], in_=z0[:], func=Act.Sin,
                                         bias=b0s[:], scale=W0_FREQ)
                    # w1t pre-scaled by 30 on host; wrap 30*z1+30*b1 into
                    # [-pi, pi] (ACT Sin spline range) before the sin
                    z1 = psB.tile([P, STAGE], f32, tag="z1", space="PSUM")
                    for c2 in range(STAGE // 512):
                        sl = slice(c2 * 512, (c2 + 1) * 512)
                        nc.tensor.matmul(z1[:, sl], w1t[:], h0[:, sl],
                                         start=True, stop=True)
                    t1 = actp.tile([P, STAGE], f32, tag="t1")
                    nc.vector.add_range_wrap(out=t1[:], in_=z1[:],
                                             shift=b1s[:],
                                             bound=float(np.pi),
                                             period=float(2 * np.pi))
                    h1 = actp.tile([P, STAGE], f16, tag="h1")
                    nc.scalar.activation(out=h1[:], in_=t1[:], func=Act.Sin)
                    o2 = psC.tile([NOUT, STAGE], f32, tag="o2", space="PSUM")
                    for c2 in range(STAGE // 512):
                        sl = slice(c2 * 512, (c2 + 1) * 512)
                        nc.tensor.matmul(o2[:, sl], w2t[:], h1[:, sl],
                                         start=True, stop=True)
                    nc.vector.tensor_copy(
                        out=outsb[:, si * STAGE:(si + 1) * STAGE], in_=o2[:])
                nc.sync.dma_start(out=out_d[:, k * CHUNK:(k + 1) * CHUNK],
                                  in_=outsb[:])

    nc.compile()
    return nc


def _host_prep(inputs):
    coords = np.ascontiguousarray(inputs["coords"], dtype=np.float32)
    emb0 = np.asarray(inputs["emb0"], dtype=np.float32)
    emb1 = np.asarray(inputs["emb1"], dtype=np.float32)
    w0 = np.asarray(inputs["w0"], dtype=np.float32)
    b0 = np.asarray(inputs["b0"], dtype=np.float32)
    w1 = np.asarray(inputs["w1"], dtype=np.float32)
    b1 = np.asarray(inputs["b1"], dtype=np.float32)
    w2 = np.asarray(inputs["w2"], dtype=np.float32)

    def upsample(emb):
        i = np.arange(RES - 1)
        w = (np.arange(UPS, dtype=np.float64) / UPS).astype(np.float32)
        t = (1.0 - w)[None, :, None] * emb[i][:, None, :] \
            + w[None, :, None] * emb[i + 1][:, None, :]
        t = t.reshape(NROWS, ED)
        pad = np.zeros((NROWS_PAD - NROWS, ED), np.float32)
        return np.concatenate([t, pad], 0).astype(np.float16)

    up0 = upsample(emb0)
    up1 = upsample(emb1)
    w0t = np.ascontiguousarray(
        w0.T.reshape(2, P, HID)).astype(np.float16)        # [c, k, m]
    w1t = np.ascontiguousarray(w1.T * W0_FREQ).astype(np.float16)
    w2t = np.ascontiguousarray(w2.T).astype(np.float16)    # [k, 3]
    b0s = (W0_FREQ * b0).reshape(P, 1).astype(np.float32)
    b1s = (W0_FREQ * b1).reshape(P, 1).astype(np.float32)

    shared = dict(up0=up0, up1=up1, w0t=w0t, w1t=w1t, w2t=w2t,
                  b0s=b0s, b1s=b1s)
    in_maps = []
    for c in range(NCORES):
        shard = np.ascontiguousarray(coords[c * B:(c + 1) * B])
        in_maps.append(dict(coords=shard, **shared))
    return in_maps


def _n_of_m():
    # gather order m = c*16 + q covers point n = (16*(c//FPC) + q)*FPC + c%FPC
    m = np.arange(B)
    q = m % 16
    c = m // 16
    return (16 * (c // FPC) + q) * FPC + (c % FPC)


last_results = None


def kernel(**inputs):
    global last_results
    from concourse.bass_utils import run_bass_kernel_spmd

    if "nc" not in _cache:
        _cache["nc"] = _build_nc()
    nc = _cache["nc"]

    in_maps = _host_prep(inputs)
    trace = bool(int(os.environ.get("KERNEL_TRACE", "0")))
    res = run_bass_kernel_spmd(nc, in_maps, core_ids=list(range(NCORES)),
                               trace=trace)
    last_results = res

    b2 = np.asarray(inputs["b2"], dtype=np.float32)
    n_of_m = _n_of_m()
    outs = []
    for c in range(NCORES):
        dev = res.results[c]["out"]                  # [3, B] in gather order
        full = np.empty((B, NOUT), np.float32)
        full[n_of_m] = dev.T
        outs.append(full)
    return np.ascontiguousarray(np.concatenate(outs, 0) + b2[None, :])
